# revision 17
# baseline (speedup 1.0000x reference)
"""CanvasEncoder (ragged LSTM over compacted canvas objects) on 8 trn2 cores.

Strategy: pure data parallel over the batch (131072 samples -> 16384/core).
Host: compact valid objects to the front (reference canvas2seq), fold the
embedding layer into the LSTM input weights, build block-diagonal stationary
weights so each matmul processes two 512-sample tiles stacked on the 128
partitions.  Device: T unrolled LSTM steps; gates via fp32r matmuls into
PSUM ([I|F|O|G] banks), sigmoid/tanh on ScalarE, cell update on VectorE.
"""

import os

import numpy as np

try:
    from concourse import bacc, tile, mybir
    from concourse.bass_utils import run_bass_kernel_spmd
except ImportError:  # fallback if site path not preinstalled
    import sys

    sys.path.insert(0, "/opt/trn_rl_repo")
    from concourse import bacc, tile, mybir
    from concourse.bass_utils import run_bass_kernel_spmd

LAST_RESULTS = None  # set per call; test harnesses may read exec_time_ns

B = 131072
N = 25
D = 4
H = 64
N_CORES = 8
B_CORE = B // N_CORES          # 16384
TILE = 512                     # samples per half-pair (one matmul free dim)
PAIRS = B_CORE // (2 * TILE)   # 16
GROUP = 2                      # pairs per DVE batch
F32 = mybir.dt.float32
F32R = mybir.dt.float32r

_PROGRAM_CACHE = {}


def _build_program(T, mm_dtype=F32R):
    nc = bacc.Bacc("TRN2", target_bir_lowering=False, debug=False,
                   num_devices=N_CORES)
    MDT = mm_dtype
    xseq = nc.dram_tensor("xseq", [T, 10, PAIRS * TILE], MDT,
                          kind="ExternalInput").ap()
    wh = nc.dram_tensor("wh", [128, 4, 128], MDT, kind="ExternalInput").ap()
    wx = nc.dram_tensor("wx", [10, 4, 128], MDT, kind="ExternalInput").ap()
    hout = nc.dram_tensor("hout", [128, PAIRS * TILE], F32,
                          kind="ExternalOutput").ap()

    ACT = mybir.ActivationFunctionType
    n_groups = PAIRS // GROUP

    with tile.TileContext(nc) as tc:
        with (
            tc.tile_pool(name="const", bufs=1) as const_pool,
            tc.tile_pool(name="state", bufs=1) as state_pool,
            tc.tile_pool(name="xin", bufs=3) as x_pool,
            tc.tile_pool(name="psum", bufs=2, space="PSUM") as psum_pool,
            tc.tile_pool(name="sifo", bufs=2) as sifo_pool,
            tc.tile_pool(name="tg", bufs=2) as tg_pool,
            tc.tile_pool(name="tc", bufs=2) as tc_pool,
            tc.tile_pool(name="scr", bufs=4) as scr_pool,
        ):
            wh_t = const_pool.tile([128, 4, 128], MDT)
            wx_t = const_pool.tile([10, 4, 128], MDT)
            nc.sync.dma_start(wh_t[:], wh[:])
            nc.sync.dma_start(wx_t[:], wx[:])

            h_bufs = []
            c_bufs = []
            for g in range(n_groups):
                hb = state_pool.tile([128, GROUP, TILE], MDT, tag=f"h{g}")
                cb = state_pool.tile([128, GROUP, TILE], F32, tag=f"c{g}")
                nc.vector.memset(hb[:].bitcast(F32), 0.0)
                nc.vector.memset(cb[:], 0.0)
                h_bufs.append(hb)
                c_bufs.append(cb)

            for t in range(T):
                for g in range(n_groups):
                    x_g = x_pool.tile([10, GROUP * TILE], MDT)
                    nc.sync.dma_start(
                        x_g[:],
                        xseq[t][:, g * GROUP * TILE:(g + 1) * GROUP * TILE])
                    hb = h_bufs[g]
                    cb = c_bufs[g]
                    sifo = sifo_pool.tile([128, GROUP, 3 * TILE], F32)
                    tg = tg_pool.tile([128, GROUP, TILE], F32)
                    for pi in range(GROUP):
                        ps = psum_pool.tile([128, 4 * TILE], F32)
                        rhs_h = hb[:, pi, :]
                        rhs_x = x_g[:, pi * TILE:(pi + 1) * TILE]
                        for bank in range(4):
                            out_ap = ps[:, bank * TILE:(bank + 1) * TILE]
                            nc.tensor.matmul(
                                out_ap,
                                wx_t[:, bank, :],
                                rhs_x,
                                start=True, stop=False,
                            )
                            nc.tensor.matmul(
                                out_ap,
                                wh_t[:, bank, :],
                                rhs_h,
                                start=False, stop=True,
                            )
                        nc.scalar.activation(sifo[:, pi, :], ps[:, 0:3 * TILE],
                                             ACT.Sigmoid)
                        nc.scalar.activation(tg[:, pi, :],
                                             ps[:, 3 * TILE:4 * TILE], ACT.Tanh)

                    s_i = sifo[:, :, 0:TILE]
                    s_f = sifo[:, :, TILE:2 * TILE]
                    s_o = sifo[:, :, 2 * TILE:3 * TILE]
                    m1 = scr_pool.tile([128, GROUP, TILE], F32, tag="m1")
                    t2 = scr_pool.tile([128, GROUP, TILE], F32, tag="t2")
                    nc.vector.tensor_mul(m1[:], s_i, tg[:])
                    nc.vector.tensor_mul(t2[:], s_f, cb[:])
                    nc.vector.tensor_add(cb[:], t2[:], m1[:])
                    tcg = tc_pool.tile([128, GROUP, TILE], F32)
                    nc.scalar.activation(tcg[:], cb[:], ACT.Tanh)
                    nc.vector.tensor_mul(hb[:], s_o, tcg[:])

            for g in range(n_groups):
                nc.sync.dma_start(
                    hout[:, g * GROUP * TILE:(g + 1) * GROUP * TILE],
                    h_bufs[g][:].bitcast(F32).rearrange("p a b -> p (a b)"),
                )
    nc.compile()
    return nc


def _prep_host(canvas, W_emb, b_emb, W_ih, W_hh, b_ih, b_hh):
    canvas = np.asarray(canvas, dtype=np.float32)
    mask = canvas.sum(axis=-1) >= 0
    k = mask.sum(axis=1)
    T = int(k.max())
    if T == 0:
        return T, None, None, None
    order = np.argsort(~mask, axis=1, kind="stable")
    seq = np.take_along_axis(canvas, order[:, :, None], axis=1)
    valid = np.take_along_axis(mask, order, axis=1)
    seq = np.where(valid[:, :, None], seq, np.float32(-1.0))
    seq = np.ascontiguousarray(seq[:, :T, :])  # (B, T, 4)

    W_ih64 = np.asarray(W_ih, dtype=np.float64)
    W_x = (W_ih64 @ np.asarray(W_emb, np.float64)).astype(np.float32)  # (256,4)
    b_tot = (W_ih64 @ np.asarray(b_emb, np.float64)
             + np.asarray(b_ih, np.float64)
             + np.asarray(b_hh, np.float64)).astype(np.float32)       # (256,)
    W_hh = np.asarray(W_hh, dtype=np.float32)

    # bank order: I, F, O, G  (torch gate layout i,f,g,o)
    gate_sl = [(0, 64), (64, 128), (192, 256), (128, 192)]
    wh = np.zeros((4, 128, 128), np.float32)
    wx = np.zeros((4, 10, 128), np.float32)
    for gi, (a, b) in enumerate(gate_sl):
        whg = W_hh[a:b, :].T  # (64 in, 64 out)
        wh[gi, 0:64, 0:64] = whg
        wh[gi, 64:128, 64:128] = whg
        wxg = W_x[a:b, :].T   # (4, 64)
        wx[gi, 0:4, 0:64] = wxg
        wx[gi, 4:8, 64:128] = wxg
        wx[gi, 8, 0:64] = b_tot[a:b]
        wx[gi, 9, 64:128] = b_tot[a:b]
    wh = np.ascontiguousarray(wh.transpose(1, 0, 2))  # (128, 4, 128)
    wx = np.ascontiguousarray(wx.transpose(1, 0, 2))  # (10, 4, 128)

    # per-core xseq: (T, 10, PAIRS*TILE) with rows [x_a(4); x_b(4); 1; 1]
    xseqs = []
    for c in range(N_CORES):
        xc = seq[c * B_CORE:(c + 1) * B_CORE]            # (B_CORE, T, 4)
        xc = xc.reshape(PAIRS, 2, TILE, T, D)
        xr = xc.transpose(3, 1, 4, 0, 2).reshape(T, 8, PAIRS * TILE)
        xd = np.ones((T, 10, PAIRS * TILE), np.float32)
        xd[:, 0:8, :] = xr
        xseqs.append(np.ascontiguousarray(xd))
    return T, xseqs, wh, wx


def kernel(**inputs):
    canvas = np.asarray(inputs["canvas"], dtype=np.float32)
    assert canvas.shape == (B, N, D), canvas.shape
    T, xseqs, wh, wx = _prep_host(
        canvas, inputs["W_emb"], inputs["b_emb"], inputs["W_ih"],
        inputs["W_hh"], inputs["b_ih"], inputs["b_hh"])
    if T == 0:
        return np.zeros((B, H), np.float32)

    key = (T,)
    if key not in _PROGRAM_CACHE:
        _PROGRAM_CACHE[key] = _build_program(T)
    nc = _PROGRAM_CACHE[key]

    in_maps = [{"xseq": xseqs[c], "wh": wh, "wx": wx} for c in range(N_CORES)]
    trace = bool(os.environ.get("BASS_LSTM_TRACE"))
    res = run_bass_kernel_spmd(nc, in_maps, list(range(N_CORES)), trace=trace)
    global LAST_RESULTS
    LAST_RESULTS = res

    out = np.empty((B, H), np.float32)
    for c in range(N_CORES):
        ho = res.results[c]["hout"]  # (128, PAIRS*TILE)
        hc = ho.reshape(2, H, PAIRS, TILE).transpose(2, 0, 3, 1)
        out[c * B_CORE:(c + 1) * B_CORE] = hc.reshape(B_CORE, H)
    return out


# revision 18
# speedup vs baseline: 1.2966x; 1.2966x over previous
"""CanvasEncoder (ragged LSTM over compacted canvas objects) on 8 trn2 cores.

Strategy: pure data parallel over the batch (131072 samples -> 16384/core).
Host: compact valid objects to the front (reference canvas2seq), fold the
embedding layer into the LSTM input weights, build block-diagonal stationary
weights so each matmul processes two 512-sample tiles stacked on the 128
partitions.  Device: T unrolled LSTM steps; gates via fp32r matmuls into
PSUM ([I|F|O|G] banks), sigmoid/tanh on ScalarE, cell update on VectorE.
"""

import os

import numpy as np

try:
    from concourse import bacc, tile, mybir
    from concourse.bass_utils import run_bass_kernel_spmd
except ImportError:  # fallback if site path not preinstalled
    import sys

    sys.path.insert(0, "/opt/trn_rl_repo")
    from concourse import bacc, tile, mybir
    from concourse.bass_utils import run_bass_kernel_spmd

LAST_RESULTS = None  # set per call; test harnesses may read exec_time_ns

B = 131072
N = 25
D = 4
H = 64
N_CORES = 8
B_CORE = B // N_CORES          # 16384
TILE = 512                     # samples per half-pair (one matmul free dim)
PAIRS = B_CORE // (2 * TILE)   # 16
GROUP = 2                      # pairs per DVE batch
F32 = mybir.dt.float32
F32R = mybir.dt.float32r
F16 = mybir.dt.float16
MM_DTYPE = os.environ.get("BASS_LSTM_MM_DTYPE", "f16")

_PROGRAM_CACHE = {}


def _build_program(T, mm_dtype):
    nc = bacc.Bacc("TRN2", target_bir_lowering=False, debug=False,
                   num_devices=N_CORES)
    MDT = mm_dtype
    xseq = nc.dram_tensor("xseq", [T, 10, PAIRS * TILE], MDT,
                          kind="ExternalInput").ap()
    wh = nc.dram_tensor("wh", [128, 4, 128], MDT, kind="ExternalInput").ap()
    wx = nc.dram_tensor("wx", [10, 4, 128], MDT, kind="ExternalInput").ap()
    hout = nc.dram_tensor("hout", [128, PAIRS * TILE], MDT,
                          kind="ExternalOutput").ap()

    ACT = mybir.ActivationFunctionType
    n_groups = PAIRS // GROUP

    with tile.TileContext(nc) as tc:
        with (
            tc.tile_pool(name="const", bufs=1) as const_pool,
            tc.tile_pool(name="state", bufs=1) as state_pool,
            tc.tile_pool(name="xin", bufs=3) as x_pool,
            tc.tile_pool(name="psum", bufs=2, space="PSUM") as psum_pool,
            tc.tile_pool(name="sifo", bufs=2) as sifo_pool,
            tc.tile_pool(name="tg", bufs=2) as tg_pool,
            tc.tile_pool(name="tc", bufs=2) as tc_pool,
            tc.tile_pool(name="scr", bufs=4) as scr_pool,
        ):
            wh_t = const_pool.tile([128, 4, 128], MDT)
            wx_t = const_pool.tile([10, 4, 128], MDT)
            nc.sync.dma_start(wh_t[:], wh[:])
            nc.sync.dma_start(wx_t[:], wx[:])

            h_bufs = []
            c_bufs = []
            for g in range(n_groups):
                hb = state_pool.tile([128, GROUP, TILE], MDT, tag=f"h{g}")
                cb = state_pool.tile([128, GROUP, TILE], F32, tag=f"c{g}")
                nc.vector.memset(hb[:].bitcast(F32 if mm_dtype != F16 else mybir.dt.uint16), 0.0)
                nc.vector.memset(cb[:], 0.0)
                h_bufs.append(hb)
                c_bufs.append(cb)

            for t in range(T):
                for g in range(n_groups):
                    x_g = x_pool.tile([10, GROUP * TILE], MDT)
                    nc.sync.dma_start(
                        x_g[:],
                        xseq[t][:, g * GROUP * TILE:(g + 1) * GROUP * TILE])
                    hb = h_bufs[g]
                    cb = c_bufs[g]
                    sifo = sifo_pool.tile([128, GROUP, 3 * TILE], F32)
                    tg = tg_pool.tile([128, GROUP, TILE], F32)
                    for pi in range(GROUP):
                        ps = psum_pool.tile([128, 4 * TILE], F32)
                        rhs_h = hb[:, pi, :]
                        rhs_x = x_g[:, pi * TILE:(pi + 1) * TILE]
                        for bank in range(4):
                            out_ap = ps[:, bank * TILE:(bank + 1) * TILE]
                            nc.tensor.matmul(
                                out_ap,
                                wx_t[:, bank, :],
                                rhs_x,
                                start=True, stop=False,
                            )
                            nc.tensor.matmul(
                                out_ap,
                                wh_t[:, bank, :],
                                rhs_h,
                                start=False, stop=True,
                            )
                        nc.scalar.activation(sifo[:, pi, :], ps[:, 0:3 * TILE],
                                             ACT.Sigmoid)
                        nc.scalar.activation(tg[:, pi, :],
                                             ps[:, 3 * TILE:4 * TILE], ACT.Tanh)

                    s_i = sifo[:, :, 0:TILE]
                    s_f = sifo[:, :, TILE:2 * TILE]
                    s_o = sifo[:, :, 2 * TILE:3 * TILE]
                    m1 = scr_pool.tile([128, GROUP, TILE], F32, tag="m1")
                    t2 = scr_pool.tile([128, GROUP, TILE], F32, tag="t2")
                    nc.vector.tensor_mul(m1[:], s_i, tg[:])
                    nc.vector.tensor_mul(t2[:], s_f, cb[:])
                    nc.vector.tensor_add(cb[:], t2[:], m1[:])
                    tcg = tc_pool.tile([128, GROUP, TILE], F32)
                    nc.scalar.activation(tcg[:], cb[:], ACT.Tanh)
                    nc.vector.tensor_mul(hb[:], s_o, tcg[:])

            for g in range(n_groups):
                nc.sync.dma_start(
                    hout[:, g * GROUP * TILE:(g + 1) * GROUP * TILE],
                    h_bufs[g][:].rearrange("p a b -> p (a b)"),
                )
    nc.compile()
    return nc


def _prep_host(canvas, W_emb, b_emb, W_ih, W_hh, b_ih, b_hh):
    canvas = np.asarray(canvas, dtype=np.float32)
    mask = canvas.sum(axis=-1) >= 0
    k = mask.sum(axis=1)
    T = int(k.max())
    if T == 0:
        return T, None, None, None
    order = np.argsort(~mask, axis=1, kind="stable")
    seq = np.take_along_axis(canvas, order[:, :, None], axis=1)
    valid = np.take_along_axis(mask, order, axis=1)
    seq = np.where(valid[:, :, None], seq, np.float32(-1.0))
    seq = np.ascontiguousarray(seq[:, :T, :])  # (B, T, 4)

    W_ih64 = np.asarray(W_ih, dtype=np.float64)
    W_x = (W_ih64 @ np.asarray(W_emb, np.float64)).astype(np.float32)  # (256,4)
    b_tot = (W_ih64 @ np.asarray(b_emb, np.float64)
             + np.asarray(b_ih, np.float64)
             + np.asarray(b_hh, np.float64)).astype(np.float32)       # (256,)
    W_hh = np.asarray(W_hh, dtype=np.float32)

    # bank order: I, F, O, G  (torch gate layout i,f,g,o)
    gate_sl = [(0, 64), (64, 128), (192, 256), (128, 192)]
    wh = np.zeros((4, 128, 128), np.float32)
    wx = np.zeros((4, 10, 128), np.float32)
    for gi, (a, b) in enumerate(gate_sl):
        whg = W_hh[a:b, :].T  # (64 in, 64 out)
        wh[gi, 0:64, 0:64] = whg
        wh[gi, 64:128, 64:128] = whg
        wxg = W_x[a:b, :].T   # (4, 64)
        wx[gi, 0:4, 0:64] = wxg
        wx[gi, 4:8, 64:128] = wxg
        wx[gi, 8, 0:64] = b_tot[a:b]
        wx[gi, 9, 64:128] = b_tot[a:b]
    wh = np.ascontiguousarray(wh.transpose(1, 0, 2))  # (128, 4, 128)
    wx = np.ascontiguousarray(wx.transpose(1, 0, 2))  # (10, 4, 128)

    # per-core xseq: (T, 10, PAIRS*TILE) with rows [x_a(4); x_b(4); 1; 1]
    xseqs = []
    for c in range(N_CORES):
        xc = seq[c * B_CORE:(c + 1) * B_CORE]            # (B_CORE, T, 4)
        xc = xc.reshape(PAIRS, 2, TILE, T, D)
        xr = xc.transpose(3, 1, 4, 0, 2).reshape(T, 8, PAIRS * TILE)
        xd = np.ones((T, 10, PAIRS * TILE), np.float32)
        xd[:, 0:8, :] = xr
        xseqs.append(np.ascontiguousarray(xd))
    return T, xseqs, wh, wx


def kernel(**inputs):
    canvas = np.asarray(inputs["canvas"], dtype=np.float32)
    assert canvas.shape == (B, N, D), canvas.shape
    T, xseqs, wh, wx = _prep_host(
        canvas, inputs["W_emb"], inputs["b_emb"], inputs["W_ih"],
        inputs["W_hh"], inputs["b_ih"], inputs["b_hh"])
    if T == 0:
        return np.zeros((B, H), np.float32)

    mdt = F16 if MM_DTYPE == "f16" else F32R
    np_mdt = np.float16 if MM_DTYPE == "f16" else np.float32
    xseqs = [x.astype(np_mdt) for x in xseqs]
    wh = wh.astype(np_mdt)
    wx = wx.astype(np_mdt)
    key = (T, MM_DTYPE)
    if key not in _PROGRAM_CACHE:
        _PROGRAM_CACHE[key] = _build_program(T, mdt)
    nc = _PROGRAM_CACHE[key]

    in_maps = [{"xseq": xseqs[c], "wh": wh, "wx": wx} for c in range(N_CORES)]
    trace = bool(os.environ.get("BASS_LSTM_TRACE"))
    res = run_bass_kernel_spmd(nc, in_maps, list(range(N_CORES)), trace=trace)
    global LAST_RESULTS
    LAST_RESULTS = res

    out = np.empty((B, H), np.float32)
    for c in range(N_CORES):
        ho = np.asarray(res.results[c]["hout"], dtype=np.float32)
        hc = ho.reshape(2, H, PAIRS, TILE).transpose(2, 0, 3, 1)
        out[c * B_CORE:(c + 1) * B_CORE] = hc.reshape(B_CORE, H)
    return out
